# revision 52
# baseline (speedup 1.0000x reference)
"""Trainium2 Bass kernel for nn_Nested_Res2Net_TDNN (B=32, CIN=1024, T=600).

Sharding: data-parallel over batch across 8 NeuronCores (4 batch elements
per core), parameters replicated.

v2: all conv matmuls run in fp8(e4m3) with MatmulPerfMode.DoubleRow, which
streams TWO 128-deep contractions per column at 0.5 cycles/col:
  - the 3 dilation-2 taps of each scale branch become 2 paired matmuls
    (taps d0+d2 share one DoubleRow op via an overlapping column AP with
    stride 4; tap d1 pairs with a resident zero plane),
  - branch states ping-pong between two fp8 planes of one [128,3,W2] tile
    (plane 2 = zeros), so the 8 conv3-accumulation terms collapse into 4
    DoubleRow pairs reading planes (0,1),
  - each tap-d1 matrix carries identity rows above the active block so the
    pristine bn1 slices propagate through the ping-pong planes for free,
  - conv1 = pair (W1 @ spx_next during the SE gap, W1'@OB after the gate)
    over a [128,3,W2] tile holding (spx8, OB8, zeros).
BN affines fold into following matmul weights as in v1 (pad cols hold
-c/k).  Post-processing (relu+bias psum->fp8) runs valid-columns-only,
alternating DVE/ACT; pad columns are written BEFORE each branch's taps
(their previous readers are two branches back), which unblocks the
branch-to-branch chunk pipeline.  The per-block +co residual shift is
pre-added into the fp16 x slices on the host; the raw spx[7] pool
contribution is computed in the prologue.  SE and the classifier stay
fp16.  (GPSIMD tensor ops and DVE scalar_tensor_tensor measured 4-9x
slower than DVE/ACT equivalents on hardware - avoid; gpsimd tensor_copy
and tensor_tensor are usable for off-critical-path work.)  The SE/fc
psums live in the branch-psum ring so it can triple-buffer (the branch
chunk pipeline depth was the binding constraint at ring depth 2); the
SE squeeze sums split 2 DVE tensor_reduce + 2 ACT identity-accum so the
per-block squeeze tail is parallel across engines.  The next block's bn1
pads/posts are emitted inside the tails, chunk-by-chunk as each conv1
psum chunk stops, so block i+1's branch chain starts without waiting for
all of block i's tail work.
"""

import sys

for _p in ("/opt/trn_rl_repo",):
    if _p not in sys.path:
        sys.path.insert(0, _p)

import numpy as np
import ml_dtypes

import concourse.bass as bass
import concourse.mybir as mybir
import concourse.tile as tile
from concourse.bass_utils import run_bass_kernel_spmd

B, CIN, T0 = 32, 1024, 600
NES, SCALE = 8, 8
CBLK, WSC = 128, 16
NIN, NOUT = 7, 7
EPS = 1e-5
NCORES = 8
BL = B // NCORES

F32 = mybir.dt.float32
F16 = mybir.dt.float16
F8 = mybir.dt.float8e4
E4 = ml_dtypes.float8_e4m3fn
ALU = mybir.AluOpType
ACTF = mybir.ActivationFunctionType
DR = mybir.MatmulPerfMode.DoubleRow

# fp8 pair-weight indices (per block, [K, 2, M] lhsT planes)
PW1A = 0          # (W1 perm, 0)            @ (spx8, zero)
PW1B = 1          # (W1 perm ko-folded, 0)  @ (OB8, zero)
PTA = 2           # 2+2j: (tap d0, tap d2) k-folded
PTB = 3           # 3+2j: (tap d1 + identity passthrough, 0)
PC3 = 16          # 16+q: conv3 pairs (x7,c3_0) (c3_1,c3_2) (c3_3,c3_4) (c3_5,c3_6)
NPW = 20
# fp16 matrices: 0 = SE fc1 (k3/T folded), 1 = SE fc2
# VEC vector indices (per block, [128, NVEC], fp32 per-partition scalars)
VB1 = 0           # bn1 relu bias (b1 perm + W1@co_prev const)
VPAD1 = 1         # -c1/k1 pad value (permuted)
VBRB = 2          # 2+2j: branch relu bias (full 128; passthrough rows 0)
VBRP = 3          # 3+2j: branch pad value (full 128; passthrough -c1/k1)
VB3 = 16          # bn3 relu bias (b3 + conv3 consts)
VK3, VC3 = 17, 18
VSE1B, VSE2B = 19, 20
VKO, VCO = 21, 22
VPKF, VPCF = 23, 24   # pool: kf*ko, kf*co+cf
NVEC = 25


def _perm():
    """ZS slice order: s=0 is spx[7] (channels 112:128), s>=1 is spx[s-1]."""
    p = np.zeros(128, np.int64)
    for s in range(8):
        for c in range(16):
            p[16 * s + c] = 112 + c if s == 0 else 16 * (s - 1) + c
    return p


def _bnkc(p):
    g, b, m, v = [np.asarray(a, np.float64) for a in p]
    k = g / np.sqrt(v + EPS)
    return k, b - m * k


def _prep(inp, T):
    f = lambda n: np.asarray(inp[n], np.float64)
    w1, b1, bn1 = f("w1"), f("b1"), f("bn1")
    cw, cb, ibn = f("cw"), f("cb"), f("ibn")
    w3, b3, bn3 = f("w3"), f("b3"), f("bn3")
    se1w, se1b = f("se1w"), f("se1b")
    se2w, se2b = f("se2w"), f("se2b")
    obn, fbn = f("obn"), f("fbn")
    fcw, fcb = f("fcw"), f("fcb")
    ws = [f(f"ws{j}") for j in range(NIN)]
    perm = _perm()
    kf, cf = _bnkc(fbn)  # (1024,)

    LW8 = np.zeros((NOUT, NPW, 2, 128, 128), np.float64)  # [i, w, plane, K, M]
    LW16 = np.zeros((NOUT, 2, 128, 128), np.float64)
    VEC = np.zeros((NOUT, NVEC, 128), np.float64)
    for i in range(NOUT):
        k1, c1 = _bnkc(bn1[i])
        k1p, c1p = k1[perm], c1[perm]
        kb, cbv = [], []
        for j in range(NIN):
            kj, cj = _bnkc(ibn[i, j])
            kb.append(kj)
            cbv.append(cj)
        assert np.abs(k1).min() > 1e-3
        assert min(np.abs(k).min() for k in kb) > 1e-3

        def row_kc(j, k1p=k1p, c1p=c1p, kb=kb, cbv=cbv):
            k = k1p.copy()
            c = c1p.copy()
            if j > 0:
                n = 16 * (j + 1)
                k[:n] = np.tile(kb[j - 1], j + 1)
                c[:n] = np.tile(cbv[j - 1], j + 1)
            return k, c

        LW8[i, PW1A, 0] = w1[i][perm, :].T
        ko, co = _bnkc(obn[i])
        if i > 0:
            ko_p, co_p = _bnkc(obn[i - 1])
            LW8[i, PW1B, 0] = (w1[i] * ko_p[None, :])[perm, :].T

        for j in range(NIN):
            krow, crow = row_kc(j)
            nact = 16 * (j + 2)
            blk = [cw[i, j, :, :, d, 0].T for d in range(3)]  # [ci, co]
            tap = [np.zeros((128, 128)) for _ in range(3)]
            bias_full = np.zeros(128)
            for s in range(j + 2):
                r0 = 16 * s
                const_s = np.zeros(16)
                for d in range(3):
                    tap[d][r0:r0 + 16, r0:r0 + 16] = blk[d] * krow[r0:r0 + 16, None]
                    const_s += blk[d].T @ crow[r0:r0 + 16]
                bias_full[r0:r0 + 16] = cb[i, j] + const_s
            for r in range(nact, 128):
                tap[1][r, r] = 1.0
            LW8[i, PTA + 2 * j, 0] = tap[0]
            LW8[i, PTA + 2 * j, 1] = tap[2]
            LW8[i, PTB + 2 * j, 0] = tap[1]
            VEC[i, VBRB + 2 * j] = bias_full  # passthrough rows stay 0
            padv = np.concatenate([np.tile(-cbv[j] / kb[j], j + 2),
                                   -c1p[nact:] / k1p[nact:]])
            VEC[i, VBRP + 2 * j] = padv

        # conv3 pairs
        c3const = np.zeros(128)
        IX7 = np.zeros((128, 128))
        IX7[0:16, :] = w3[i][:, 112:128].T * k1p[0:16, None]
        c3const += w3[i][:, 112:128] @ c1p[0:16]
        IC3 = []
        for j in range(NIN):
            wj = ws[j][i]
            M_ = np.zeros((128, 128))
            for s in range(j + 2):
                r0 = 16 * s
                M_[r0:r0 + 16, :] = (wj[s] * w3[i][:, 16 * j:16 * j + 16].T) * kb[j][:, None]
                c3const += wj[s] * (w3[i][:, 16 * j:16 * j + 16] @ cbv[j])
            IC3.append(M_)
        LW8[i, PC3 + 0, 0], LW8[i, PC3 + 0, 1] = IX7, IC3[0]
        LW8[i, PC3 + 1, 0], LW8[i, PC3 + 1, 1] = IC3[1], IC3[2]
        LW8[i, PC3 + 2, 0], LW8[i, PC3 + 2, 1] = IC3[3], IC3[4]
        LW8[i, PC3 + 3, 0], LW8[i, PC3 + 3, 1] = IC3[5], IC3[6]

        k3, c3 = _bnkc(bn3[i])
        LW16[i, 0, :, 0:16] = (se1w[i] * k3[None, :] / T).T
        LW16[i, 1, 0:16, :] = se2w[i].T

        VEC[i, VB1] = b1[i][perm]
        if i > 0:
            _, co_p = _bnkc(obn[i - 1])
            VEC[i, VB1] += (w1[i] @ co_p)[perm]
        VEC[i, VPAD1] = -c1p / k1p
        VEC[i, VB3] = b3[i] + c3const
        VEC[i, VK3], VEC[i, VC3] = k3, c3
        VEC[i, VSE1B, :16] = se1b[i] + se1w[i] @ c3
        VEC[i, VSE2B] = se2b[i]
        VEC[i, VKO], VEC[i, VCO] = ko, co
        kfi = kf[128 * i:128 * (i + 1)]
        cfi = cf[128 * i:128 * (i + 1)]
        VEC[i, VPKF] = kfi * ko
        VEC[i, VPCF] = kfi * co + cfi

    vec7 = np.stack([kf[896:1024], cf[896:1024]], axis=1)  # [128, 2]
    fcwp = np.zeros((128, 8, 2))
    for g in range(8):
        fcwp[:, g, :] = (fcw[:, 128 * g:128 * (g + 1)] / T).T
    lw8 = np.ascontiguousarray(
        LW8.transpose(0, 3, 1, 2, 4).reshape(NOUT, 128, NPW * 2 * 128).astype(E4))
    lw16 = np.ascontiguousarray(
        LW16.transpose(0, 2, 1, 3).reshape(NOUT, 128, 2 * 128).astype(np.float16))
    vecf = np.ascontiguousarray(VEC.transpose(0, 2, 1).astype(np.float32))

    # fp16 x with per-block +co_[i-1] folded into slices 1..6 (residual path);
    # slices 0 and 7 stay raw
    x = f("x")
    x16 = x.copy()
    for i in range(NOUT - 1):  # nxtco_i = x_{i+1} + co_i, i = 0..5
        _, co_i = _bnkc(obn[i])
        x16[:, 128 * (i + 1):128 * (i + 2), :] += co_i[None, :, None]
    return (lw8, lw16, vecf,
            np.ascontiguousarray(x.astype(E4)),
            np.ascontiguousarray(x16.astype(np.float16)),
            np.ascontiguousarray(vec7.astype(np.float32)),
            np.ascontiguousarray(fcwp.reshape(128, 16).astype(np.float16)),
            np.ascontiguousarray(fcb.reshape(2, 1).astype(np.float32)))


def _split_waits(nc, max_waits=1):
    """walrus's TRN2 codegen rejects >1 sync wait on drain/matmul (and
    possibly other) instructions; peel extras onto preceding single-wait
    no-ops on the same engine."""
    n_new = 0
    for fn in nc.m.functions:
        for bb in fn.blocks:
            out = []
            for ins in bb.instructions:
                si = ins.sync_info
                if si is not None and len(si.on_wait) > max_waits:
                    waits = list(si.on_wait)
                    for w in waits[max_waits:]:
                        nop = mybir.InstNoOp(
                            name=f"I-splitwait-{n_new}",
                            sync_info=mybir.SyncInfo(on_wait=[w], on_update=[]),
                            bass_nofuse=True,
                            engine=ins.engine,
                        )
                        out.append(nop)
                        n_new += 1
                    ins.sync_info = mybir.SyncInfo(
                        on_wait=waits[:max_waits], on_update=list(si.on_update))
                out.append(ins)
            bb.instructions = out
    return n_new


def build(T=T0):
    nc = bass.Bass("TRN2")
    P = T + 4
    W2 = BL * P + 4
    lo, hi = 2, 2 + BL * P
    chunks = []
    c = lo
    while c < hi:
        w = min(512, hi - c)
        chunks.append((c, w))
        c += w
    NCH = len(chunks)
    pieces = []
    for b in range(BL):
        v0, v1 = 4 + P * b, 4 + P * b + T
        for ci, (c0, w) in enumerate(chunks):
            s, e = max(v0, c0), min(v1, c0 + w)
            if s < e:
                pieces.append((ci, s, e - s, b))
    batch_pieces = [[k for k, p in enumerate(pieces) if p[3] == b]
                    for b in range(BL)]
    # last tail-emitted piece per chunk (tails go b=0..BL-1 in order)
    last_piece_of_chunk = {}
    for k, (ci, s, w, b) in enumerate(pieces):
        last_piece_of_chunk[ci] = k
    NP = len(pieces)

    x8_d = nc.dram_tensor("x8", [BL, CIN, T], F8, kind="ExternalInput")
    x16_d = nc.dram_tensor("x16", [BL, CIN, T], F16, kind="ExternalInput")
    lw8_d = nc.dram_tensor("lw8", [NOUT, 128, NPW * 2 * 128], F8, kind="ExternalInput")
    lw16_d = nc.dram_tensor("lw16", [NOUT, 128, 2 * 128], F16, kind="ExternalInput")
    vec_d = nc.dram_tensor("vec", [NOUT, 128, NVEC], F32, kind="ExternalInput")
    vec7_d = nc.dram_tensor("vec7", [128, 2], F32, kind="ExternalInput")
    fcw_d = nc.dram_tensor("fcw", [128, 16], F16, kind="ExternalInput")
    fcb_d = nc.dram_tensor("fcb", [2, 1], F32, kind="ExternalInput")
    y_d = nc.dram_tensor("y", [2, BL], F32, kind="ExternalOutput")

    with tile.TileContext(nc) as tc:
        with tc.tile_pool(name="state", bufs=1) as state, \
             tc.tile_pool(name="wp", bufs=2) as wp, \
             tc.tile_pool(name="vp", bufs=2) as vp, \
             tc.tile_pool(name="bpsum", bufs=3, space="PSUM") as bpsum, \
             tc.tile_pool(name="cpsum", bufs=NCH, space="PSUM") as cpsum:

            Z3 = state.tile([128, 3, W2], F8, tag="Z3")
            XP = state.tile([128, 3, W2], F8, tag="XP")
            X16 = [state.tile([128, W2], F16, tag=f"X16{q}", name=f"X16{q}")
                   for q in range(2)]
            X7 = state.tile([128, W2], F16, tag="X7")
            O3 = state.tile([128, W2], F16, tag="O3")
            Rb = state.tile([128, W2], F16, tag="Rb")
            scr = state.tile([128, T], F16, tag="scr")
            scrD = state.tile([128, T], F16, tag="scrD")
            Mt = state.tile([128, 8 * BL], F32, tag="Mt")
            Mt_r = state.tile([128, 8 * BL], F16, tag="Mt_r")
            sq32 = state.tile([128, BL], F32, tag="sq32")
            youts = state.tile([2, BL], F32, tag="youts")
            sq_r = state.tile([128, BL], F16, tag="sq_r")
            seh = state.tile([128, BL], F16, tag="seh")
            gate = state.tile([128, BL], F32, tag="gate")
            kc3g = state.tile([128, 2, BL], F32, tag="kc3g")
            k3g = kc3g[:, 0, :]
            c3g = kc3g[:, 1, :]
            outs = state.tile([2, BL], F32, tag="outs")
            fcw_s = state.tile([128, 16], F16, tag="fcw")
            fcb_s = state.tile([2, 1], F32, tag="fcb")
            vec7_s = state.tile([128, 2], F32, tag="vec7")

            # Z3: zero plane (2) fully; guard cols of planes 0/1 (outside the
            # chunk grid, read by shifted taps).  XP: spx plane 0 (its pads
            # are read by full-chunk conv1 matmuls) + zero plane 2.  All
            # other buffers are written before any read of the same region.
            nc.gpsimd.memset(Z3[:, 2, :], 0)
            nc.gpsimd.memset(XP[:, 2, :], 0)
            nc.vector.memset(Z3[:, 0:2, 0:2], 0)
            nc.vector.memset(Z3[:, 0:2, W2 - 2:W2], 0)
            # XP plane 0: only pad/guard cols need zeroing (valid cols DMA'd)
            nc.vector.memset(XP[:, 0, 0:2], 0)
            nc.vector.memset(XP[:, 0, W2 - 2:W2], 0)
            vxp = XP[:, 0, lo:hi].rearrange("p (b q) -> p b q", q=P)
            vv0 = vxp[:, :, 0:2]
            vv0.ap.insert(2, [T + 2, 2])
            nc.vector.memset(vv0, 0)



            Z3f = Z3.rearrange("p a b -> p (a b)")

            def tapA(src, c0, w):
                a = Z3f[:, src * W2 + c0 - 2: src * W2 + c0 - 2 + w]
                a.ap.insert(1, [4, 2])
                return a

            def tapB(src, c0, w):
                if src == 0:
                    return Z3[:, 0:3:2, c0:c0 + w]
                return Z3[:, 1:3, c0:c0 + w]

            def load_x8(i):
                v = XP[:, 0, lo:hi].rearrange("p (b q) -> p b q", q=P)
                nc.sync.dma_start(
                    out=v[:, :, 2:T + 2],
                    in_=x8_d[:, 128 * i:128 * (i + 1), :].rearrange("b c t -> c b t"))

            def load_x16(i):
                v = X16[i % 2][:, lo:hi].rearrange("p (b q) -> p b q", q=P)
                nc.sync.dma_start(
                    out=v[:, :, 2:T + 2],
                    in_=x16_d[:, 128 * i:128 * (i + 1), :].rearrange("b c t -> c b t"))

            def load_w(i):
                t8 = wp.tile([128, NPW, 2, 128], F8, tag="lw8")
                nc.sync.dma_start(
                    out=t8[:], in_=lw8_d[i].rearrange("k (w two m) -> k w two m",
                                                      two=2, m=128))
                t16 = wp.tile([128, 2, 128], F16, tag="lw16")
                nc.sync.dma_start(
                    out=t16[:], in_=lw16_d[i].rearrange("k (two m) -> k two m", m=128))
                v = vp.tile([128, NVEC], F32, tag="vec")
                nc.sync.dma_start(out=v[:], in_=vec_d[i])
                return t8, t16, v

            def pad_write(plane_ap, vecap, eng):
                v = plane_ap[:, lo:hi].rearrange("p (b q) -> p b q", q=P)
                vv = v[:, :, 0:2]
                vv.ap.insert(2, [T + 2, 2])  # cols {0,1,T+2,T+3} per batch
                bc = vecap[:, :, None, None].to_broadcast([128, BL, 2, 2])
                eng.tensor_copy(out=vv, in_=bc)

            def post_set(dst_plane_ap, psl, biasap, set_idx):
                """relu(psum + bias) -> fp8 plane over valid pieces only
                (pads are written separately, BEFORE the taps), DVE/ACT split
                (GPSIMD cannot read PSUM)."""
                for kpc, (ci, s, w, b) in enumerate(pieces):
                    c0 = chunks[ci][0]
                    ps = psl[ci][:, s - c0:s - c0 + w]
                    o = dst_plane_ap[:, s:s + w]
                    if (set_idx + kpc) % 2 == 0:
                        nc.vector.tensor_scalar(o, ps, biasap, 0.0,
                                                ALU.add, ALU.max)
                    else:
                        nc.scalar.activation(o, ps, ACTF.Relu,
                                             bias=biasap, scale=1.0)

            nc.vector.memset(youts[:], 0)

            def fc_partial(g):
                nc.vector.tensor_copy(out=Mt_r[:, BL * g:BL * (g + 1)],
                                      in_=Mt[:, BL * g:BL * (g + 1)])
                fps = bpsum.tile([2, BL], F32, tag="bps", name="fps")
                nc.tensor.matmul(fps, fcw_s[:, 2 * g:2 * g + 2],
                                 Mt_r[:, BL * g:BL * (g + 1)],
                                 start=True, stop=True)
                nc.vector.tensor_tensor(youts[:], youts[:], fps[:], ALU.add)

            lw_t, lw16_t, Vt = load_w(0)
            load_x8(0)
            load_x16(0)
            # small constant loads after the critical block-0 transfers
            nc.sync.dma_start(out=fcw_s[:], in_=fcw_d[:])
            nc.sync.dma_start(out=fcb_s[:], in_=fcb_d[:])
            nc.sync.dma_start(out=vec7_s[:], in_=vec7_d[:])
            # raw spx[7]: load + pool its final-classifier contribution up
            # front (independent of the block chain; fills the cold start)
            v7 = X7[:, lo:hi].rearrange("p (b q) -> p b q", q=P)
            nc.sync.dma_start(
                out=v7[:, :, 2:T + 2],
                in_=x16_d[:, 896:1024, :].rearrange("b c t -> c b t"))
            for b in range(BL):
                v0 = 4 + P * b
                nc.scalar.activation(scr[:], X7[:, v0:v0 + T], ACTF.Relu,
                                     bias=vec7_s[:, 1:2], scale=vec7_s[:, 0:1],
                                     accum_out=Mt[:, BL * 7 + b:BL * 7 + b + 1])
            fc_partial(7)

            # prologue: block-0 conv1 (spx part only; OB plane is zeros anyway)
            c1ps = [cpsum.tile([128, 512], F32, tag="cps", name="c1ps")[:, :w]
                    for (c0, w) in chunks]
            for k, (c0, w) in enumerate(chunks):
                nc.tensor.matmul(c1ps[k], lw_t[:, PW1A], XP[:, 0:3:2, c0:c0 + w],
                                 start=True, stop=True, perf_mode=DR)

            for i in range(NOUT):
                nW = load_w(i + 1) if i < NOUT - 1 else None
                if i < NOUT - 1:
                    load_x8(i + 1)
                    load_x16(i + 1)

                # bn1 relu -> z_init in plane 0.  For i>0 these were
                # already emitted inside block i-1's tails (chunk-by-chunk as
                # the conv1-late pairs stopped); only block 0 does them here.
                if i == 0:
                    pad_write(Z3[:, 0, :], Vt[:, VPAD1:VPAD1 + 1], nc.gpsimd)
                    post_set(Z3[:, 0, :], c1ps, Vt[:, VB1:VB1 + 1], set_idx=0)

                cps = [cpsum.tile([128, 512], F32, tag="cps", name="cps")[:, :w]
                       for (c0, w) in chunks]

                for j in range(NIN):
                    src = j % 2
                    dst = 1 - src
                    # dst-plane pads first: their previous readers (branch
                    # j-1's taps) are already issued, and branch j's taps
                    # don't touch plane dst -- so this only gates branch j+1
                    if j < NIN - 1:
                        pad_write(Z3[:, dst, :],
                                  Vt[:, VBRP + 2 * j:VBRP + 2 * j + 1],
                                  nc.gpsimd)
                    bps = [bpsum.tile([128, 512], F32, tag="bps", name="bps")[:, :w]
                           for (c0, w) in chunks]
                    for k, (c0, w) in enumerate(chunks):
                        nc.tensor.matmul(bps[k], lw_t[:, PTA + 2 * j],
                                         tapA(src, c0, w),
                                         start=True, stop=False, perf_mode=DR)
                        nc.tensor.matmul(bps[k], lw_t[:, PTB + 2 * j],
                                         tapB(src, c0, w),
                                         start=False, stop=True, perf_mode=DR)
                    post_set(Z3[:, dst, :], bps,
                             Vt[:, VBRB + 2 * j:VBRB + 2 * j + 1], set_idx=j + 1)
                    if j % 2 == 0:
                        q = j // 2
                        for k, (c0, w) in enumerate(chunks):
                            nc.tensor.matmul(cps[k], lw_t[:, PC3 + q],
                                             Z3[:, 0:2, c0:c0 + w],
                                             start=(j == 0), stop=(j == NIN - 1),
                                             perf_mode=DR)

                # bn3 relu -> O3 (fp16); SE squeeze via DVE reduces
                # (faster than the serial ACT accum_out chain)
                post_set(O3, cps, Vt[:, VB3:VB3 + 1], set_idx=8)
                for b in range(BL):
                    v0 = 4 + P * b
                    if b < 2:
                        nc.vector.tensor_reduce(sq32[:, b:b + 1],
                                                O3[:, v0:v0 + T],
                                                mybir.AxisListType.X, ALU.add)
                    else:
                        nc.scalar.activation(scrD[:], O3[:, v0:v0 + T],
                                             ACTF.Identity, scale=1.0,
                                             accum_out=sq32[:, b:b + 1])
                nc.vector.tensor_copy(out=sq_r[:], in_=sq32[:])

                # next block's conv1 spx-part fills the SE-gap on the PE
                if nW is not None:
                    c1ps = [cpsum.tile([128, 512], F32, tag="cps",
                                       name="c1ps")[:, :w] for (c0, w) in chunks]
                    for k, (c0, w) in enumerate(chunks):
                        nc.tensor.matmul(c1ps[k], nW[0][:, PW1A],
                                         XP[:, 0:3:2, c0:c0 + w],
                                         start=True, stop=False, perf_mode=DR)

                # SE squeeze/excite (fp16)
                ps1 = bpsum.tile([128, BL], F32, tag="bps", name="ps1")
                nc.tensor.matmul(ps1, lw16_t[:, 0], sq_r[:], start=True, stop=True)
                nc.scalar.activation(seh[:], ps1, ACTF.Relu,
                                     bias=Vt[:, VSE1B:VSE1B + 1], scale=1.0)
                ps2 = bpsum.tile([128, BL], F32, tag="bps", name="ps2")
                nc.tensor.matmul(ps2, lw16_t[:, 1], seh[:], start=True, stop=True)
                nc.scalar.activation(gate[:], ps2, ACTF.Sigmoid,
                                     bias=Vt[:, VSE2B:VSE2B + 1], scale=1.0)
                nc.vector.tensor_tensor(
                    kc3g[:], gate[:, None, :].to_broadcast([128, 2, BL]),
                    Vt[:, VK3:VC3 + 1][:, :, None].to_broadcast([128, 2, BL]),
                    ALU.mult)

                # tails: OB8 = relu(o3*k3g + c3g + res) in fp8 (feeds next
                # conv1 + pool); Rb = relu(.)*ko + (x_next + co).  Next
                # block's bn1 pads/posts are emitted chunk-by-chunk as the
                # conv1 psum chunks stop, overlapping the remaining tails.
                stop_batch = {ci: pieces[kl][3]
                              for ci, kl in last_piece_of_chunk.items()}
                if nW is not None:
                    pad_write(Z3[:, 0, :], nW[2][:, VPAD1:VPAD1 + 1],
                              nc.gpsimd)
                for b in range(BL):
                    v0 = 4 + P * b
                    o3b = O3[:, v0:v0 + T]
                    res = (X16[0] if i == 0 else Rb)[:, v0:v0 + T]
                    nc.vector.tensor_scalar(o3b, o3b, k3g[:, b:b + 1],
                                            c3g[:, b:b + 1], ALU.mult, ALU.add)
                    nc.vector.tensor_tensor(o3b, o3b, res, ALU.add)
                    ob8 = XP[:, 1, v0:v0 + T]
                    if i == NOUT - 1:
                        nc.vector.tensor_scalar(ob8, o3b, 0.0, 1.0,
                                                ALU.max, ALU.mult)
                    else:
                        nc.scalar.activation(ob8, o3b, ACTF.Relu)
                    col = BL * i + b
                    nc.scalar.activation(scr[:], ob8, ACTF.Relu,
                                         bias=Vt[:, VPCF:VPCF + 1],
                                         scale=Vt[:, VPKF:VPKF + 1],
                                         accum_out=Mt[:, col:col + 1])
                    if i < NOUT - 1:
                        rb = Rb[:, v0:v0 + T]
                        nc.vector.tensor_scalar(rb, o3b, 0.0,
                                                Vt[:, VKO:VKO + 1],
                                                ALU.max, ALU.mult)
                        nc.gpsimd.tensor_tensor(
                            rb, rb, X16[(i + 1) % 2][:, v0:v0 + T], ALU.add)
                        for kpc in batch_pieces[b]:
                            ci, s, w, _ = pieces[kpc]
                            c0 = chunks[ci][0]
                            nc.tensor.matmul(
                                c1ps[ci][:, s - c0:s - c0 + w],
                                nW[0][:, PW1B], XP[:, 1:3, s:s + w],
                                start=False,
                                stop=(last_piece_of_chunk[ci] == kpc),
                                perf_mode=DR)
                        for ci2 in range(NCH):
                            if stop_batch[ci2] != b:
                                continue
                            c02 = chunks[ci2][0]
                            for kpc2, (pci, s2, pw2, pb) in enumerate(pieces):
                                if pci != ci2:
                                    continue
                                ps2 = c1ps[ci2][:, s2 - c02:s2 - c02 + pw2]
                                o2 = Z3[:, 0, s2:s2 + pw2]
                                bias2 = nW[2][:, VB1:VB1 + 1]
                                if kpc2 % 2 == 0:
                                    nc.vector.tensor_scalar(
                                        o2, ps2, bias2, 0.0, ALU.add, ALU.max)
                                else:
                                    nc.scalar.activation(
                                        o2, ps2, ACTF.Relu, bias=bias2,
                                        scale=1.0)

                fc_partial(i)
                if nW is not None:
                    lw_t, lw16_t, Vt = nW

            # classifier partials were accumulated per block into youts
            nc.scalar.activation(outs[:], youts[:], ACTF.Identity,
                                 bias=fcb_s[:], scale=1.0)
            nc.sync.dma_start(out=y_d[:], in_=outs[:])

    return nc


_NC_CACHE = {}


def _get_nc(T):
    if T not in _NC_CACHE:
        nc = build(T)
        _split_waits(nc)
        _NC_CACHE[T] = nc
    return _NC_CACHE[T]


def make_in_maps(inputs):
    x = np.asarray(inputs["x"], np.float32)
    T = x.shape[2]
    lw8, lw16, vec, x8, x16, vec7, fcw, fcb = _prep(inputs, T)
    in_maps = []
    for core in range(NCORES):
        sl = slice(core * BL, (core + 1) * BL)
        in_maps.append({
            "x8": np.ascontiguousarray(x8[sl]),
            "x16": np.ascontiguousarray(x16[sl]),
            "lw8": lw8, "lw16": lw16, "vec": vec,
            "vec7": vec7, "fcw": fcw, "fcb": fcb,
        })
    return in_maps, T


def kernel(**inputs):
    in_maps, T = make_in_maps(inputs)
    nc = _get_nc(T)
    res = run_bass_kernel_spmd(nc, in_maps, list(range(NCORES)))
    out = np.concatenate(
        [np.asarray(res.results[c]["y"]).T for c in range(NCORES)], axis=0)
    return np.ascontiguousarray(out.astype(np.float32))
